# revision 1
# baseline (speedup 1.0000x reference)
"""Trainium2 Bass kernel for nn_JointLearningModel (coref-style joint model).

Sharding: the 384x384 pair grid is split by rows across 8 NeuronCores.
Mention representations are computed on the host (pure gathers) and
uploaded replicated in transposed [H, N] layout; params replicated; the
scalar loss is computed per-core over its row block (+ its slice of the
character CE) and summed on the host.

Key optimizations over the naive version:
- The causal mask means row i only needs pair columns j < i. Rows are
  dealt to cores by column-chunk class (1, 2 or 3 x128 chunks) so all
  cores run the identical SPMD instruction stream but skip ~1/3 of the
  pair-MLP work.
- The dominant W_pair2 matmul (and the W_pair3 reduction) run in fp8
  (e4m3, weights pre-scaled by 16) with DoubleRow perf mode: 2x PE
  throughput, contraction 256 per instruction.
- Score rows move PSUM->SBUF via DMA instead of vector/scalar copies.
"""

import numpy as np
import ml_dtypes

import concourse.bass as bass
import concourse.mybir as mybir
import concourse.tile as tile
from concourse import bacc
from concourse.bass_utils import run_bass_kernel_spmd

F32 = mybir.dt.float32
BF16 = mybir.dt.bfloat16
F8 = mybir.dt.float8e4
I32 = mybir.dt.int32
AF = mybir.ActivationFunctionType
OP = mybir.AluOpType
DR = mybir.MatmulPerfMode.DoubleRow

B, L, H, M = 8, 512, 768, 383
N = M + 1          # 384 rows/cols of the pair grid
NC_ = 8            # cores
R = N // NC_       # 48 rows per core
HC = H // 128      # 6 k-chunks of the hidden dim
NEG = -10000.0
S2 = 16.0          # fp8 pre-scale on W_pair2
S3 = 16.0          # fp8 pre-scale on W_pair3

_CACHE = {}
LAST_RESULT = None


def _build_program(
    reps=1, fuse_relu=True, h1_gp=0, h1_bufs=12, h2t_bufs=6, copy_mode="act"
):
    nc = bacc.Bacc(
        "TRN2", target_bir_lowering=False, debug=False, enable_asserts=False
    )

    def din(name, shape, dt):
        return nc.dram_tensor(name, list(shape), dt, kind="ExternalInput")

    # mention representations (host-gathered), transposed layouts
    reps8 = din("reps8", [128, HC, N], F8)      # reps8[p,c,j] = reps[j, 128c+p]
    repsTl = din("repsTl", [128, HC, R], BF16)  # local rows, slot order
    repsTl8 = din("repsTl8", [128, HC, R], F8)
    # pair MLP weights (fp8, pre-scaled by S2)
    wa8 = din("wa8", [128, HC, H], F8)          # wa8[p,ci,o] = Wa[o, 128ci+p]*S2
    wb8 = din("wb8", [128, HC, H], F8)
    w28 = din("w28", [128, HC, H // 2], F8)     # W2.T * S2, fp8
    # inner dim padded to 16 so the DoubleRow k-pair stride is 16B-aligned
    w38 = din("w38", [128, 3, 16], F8)          # W3 * S3 in [:, :, 0], fp8
    b1c = din("b1c", [128, HC], F32)
    b2c = din("b2c", [128, 3], F32)
    # mention-score MLP
    wm18 = din("wm18", [128, HC, H // 2], F8)   # W_m1.T * S2, fp8
    bm1c = din("bm1c", [128, 3], F32)
    wm2T = din("wm2T", [128, 3, H // 4], BF16)
    bm2c = din("bm2c", [128, 2], F32)
    wm3c = din("wm3c", [128, 2], BF16)
    # character head
    wc1T = din("wc1T", [128, HC, H // 2], BF16)
    bc1c = din("bc1c", [128, 3], F32)
    wc2T = din("wc2T", [128, 3, 18], BF16)
    bc2r = din("bc2r", [1, 18], F32)
    # per-core loss plumbing (rows in slot order)
    maskb = din("maskb", [R, N], F32)
    multb = din("multb", [R, N], F32)
    wnll = din("wnll", [R, 1], F32)
    oneh = din("oneh", [R, 18], F32)
    wch = din("wch", [R, 1], F32)
    zrow = din("zrow", [R, N], F32)             # zeros, pre-clears sblk

    loss = nc.dram_tensor("loss", [1, 1], F32, kind="ExternalOutput")

    opts = dict(fuse_relu=fuse_relu, h1_gp=h1_gp, h1_bufs=h1_bufs,
                h2t_bufs=h2t_bufs, copy_mode=copy_mode)
    with tile.TileContext(nc) as tc:
        for rep in range(reps):
            _build_rep(nc, tc, rep, opts, dict(
                reps8=reps8, repsTl=repsTl, repsTl8=repsTl8, wa8=wa8,
                wb8=wb8, w28=w28, w38=w38, b1c=b1c, b2c=b2c, wm18=wm18,
                bm1c=bm1c, wm2T=wm2T, bm2c=bm2c, wm3c=wm3c, wc1T=wc1T,
                bc1c=bc1c, wc2T=wc2T, bc2r=bc2r, maskb=maskb,
                multb=multb, wnll=wnll, oneh=oneh, wch=wch, zrow=zrow,
                loss=loss,
            ))

    nc.compile()
    return nc


def _build_rep(nc, tc, rep, opts, io):
    fuse_relu = opts["fuse_relu"]
    with tc.tile_pool(name=f"const{rep}", bufs=1) as cp:
        def load(name, h, q=None):
            t = cp.tile(list(h.shape), h.dtype, name=f"{name}{rep}")
            (q or nc.sync).dma_start(out=t[:], in_=h.ap())
            return t

        # split the loads over both HWDGE queues (SP + ACT) so the A.T
        # and Bb input chains start in parallel
        reps8_sb = load("reps8_sb", io["reps8"])
        wa8_sb = load("wa8_sb", io["wa8"])
        wb8_sb = load("wb8_sb", io["wb8"], q=nc.scalar)
        repsTl_sb = load("repsTl_sb", io["repsTl"])
        repsTl8_sb = load("repsTl8_sb", io["repsTl8"], q=nc.scalar)
        w28_sb = load("w28_sb", io["w28"])
        w38_sb = load("w38_sb", io["w38"])
        b1c_sb = load("b1c_sb", io["b1c"], q=nc.scalar)
        b2c_sb = None if fuse_relu else load("b2c_sb", io["b2c"])
        wm18_sb = load("wm18_sb", io["wm18"])
        bm1c_sb = load("bm1c_sb", io["bm1c"])
        wm2T_sb = load("wm2T_sb", io["wm2T"])
        bm2c_sb = load("bm2c_sb", io["bm2c"])
        wm3c_sb = load("wm3c_sb", io["wm3c"])
        wc1T_sb = load("wc1T_sb", io["wc1T"], q=nc.scalar)
        bc1c_sb = load("bc1c_sb", io["bc1c"], q=nc.scalar)
        wc2T_sb = load("wc2T_sb", io["wc2T"], q=nc.scalar)
        bc2r_sb = load("bc2r_sb", io["bc2r"], q=nc.scalar)
        maskb_sb = load("maskb_sb", io["maskb"], q=nc.scalar)
        multb_sb = load("multb_sb", io["multb"], q=nc.scalar)
        wnll_sb = load("wnll_sb", io["wnll"], q=nc.scalar)
        oneh_sb = load("oneh_sb", io["oneh"], q=nc.scalar)
        wch_sb = load("wch_sb", io["wch"], q=nc.scalar)

        one1 = cp.tile([1, R], F32, name=f"one1{rep}")
        nc.vector.memset(one1[:], 1.0)

        # outputs of the preamble, used by the main loop / epilogue
        at_sb = cp.tile([128, HC, N], BF16, name=f"at_sb{rep}")   # A.T
        bb_sb = cp.tile([128, HC, R], F32, name=f"bb_sb{rep}")    # Bm.T + b1
        mskms = cp.tile([R, N], F32, name=f"mskms{rep}")          # mask+ms[j]
        sblkf = cp.tile([1, R, N], F32, name=f"sblkf{rep}")       # scores flat
        # Destination of the reshape; pre-zeroed (fast: spreads over 48
        # partitions) so rows' uncomputed masked tails read as 0.
        sblk = cp.tile([R, N], F32, name=f"sblk{rep}")
        nc.sync.dma_start(out=sblk[:], in_=io["zrow"].ap())

        # ---------- preamble matmuls: A.T, Bb, ms, mask+ms ----------
        # A.T and Bb are emitted chunk-interleaved so chunk 0 of both is
        # ready early and the first main-loop h1 op can start sooner.
        with tc.tile_pool(name=f"pre_ps{rep}", bufs=2, space="PSUM") as pp:
            for co in range(HC):
                pb = pp.tile([128, R], F32, tag="small", name=f"pb{rep}_{co}")
                for ci in range(HC):
                    nc.tensor.matmul(
                        out=pb[:],
                        lhsT=wb8_sb[:, ci, co * 128 : (co + 1) * 128],
                        rhs=repsTl8_sb[:, ci, :],
                        start=(ci == 0),
                        stop=(ci == HC - 1),
                    )
                nc.vector.tensor_scalar(
                    out=bb_sb[:, co, :],
                    in0=pb[:],
                    scalar1=1.0 / S2,
                    scalar2=b1c_sb[:, co : co + 1],
                    op0=OP.mult,
                    op1=OP.add,
                )
                pa = pp.tile([128, N], F32, tag="big", name=f"pa{rep}_{co}")
                for ci in range(0, HC, 2):
                    nc.tensor.matmul(
                        out=pa[:],
                        lhsT=wa8_sb[:, ci : ci + 2, co * 128 : (co + 1) * 128],
                        rhs=reps8_sb[:, ci : ci + 2, :],
                        start=(ci == 0),
                        stop=(ci == HC - 2),
                        perf_mode=DR,
                    )
                nc.scalar.mul(out=at_sb[:, co, :], in_=pa[:], mul=1.0 / S2)
            # mention score MLP (768 -> 384 -> 192 -> 1)
            ms1 = cp.tile([128, 3, N], BF16, name=f"ms1{rep}")
            for co in range(3):
                pm = pp.tile([128, N], F32, tag="big", name=f"pm{rep}_{co}")
                for ci in range(0, HC, 2):
                    nc.tensor.matmul(
                        out=pm[:],
                        lhsT=wm18_sb[:, ci : ci + 2, co * 128 : (co + 1) * 128],
                        rhs=reps8_sb[:, ci : ci + 2, :],
                        start=(ci == 0),
                        stop=(ci == HC - 2),
                        perf_mode=DR,
                    )
                nc.scalar.activation(
                    out=ms1[:, co, :],
                    in_=pm[:],
                    func=AF.Relu,
                    bias=bm1c_sb[:, co : co + 1],
                    scale=1.0 / S2,
                )
            ms2 = cp.tile([128, 2, N], BF16, name=f"ms2{rep}")
            for co, sz in enumerate((128, 64)):
                pm2 = pp.tile([128, N], F32, tag="big", name=f"pm2{rep}_{co}")
                for ci in range(3):
                    nc.tensor.matmul(
                        out=pm2[:sz, :],
                        lhsT=wm2T_sb[:, ci, co * 128 : co * 128 + sz],
                        rhs=ms1[:, ci, :],
                        start=(ci == 0),
                        stop=(ci == 2),
                    )
                nc.scalar.activation(
                    out=ms2[:sz, co, :],
                    in_=pm2[:sz, :],
                    func=AF.Relu,
                    bias=bm2c_sb[:sz, co : co + 1],
                )
            pms = pp.tile([1, N], F32, tag="small", name=f"pms{rep}")
            nc.tensor.matmul(
                out=pms[:], lhsT=wm3c_sb[:, 0:1], rhs=ms2[:, 0, :],
                start=True, stop=False,
            )
            nc.tensor.matmul(
                out=pms[:], lhsT=wm3c_sb[:64, 1:2], rhs=ms2[:64, 1, :],
                start=False, stop=True,
            )
            ms_sb = cp.tile([1, N], F32, name=f"ms_sb{rep}")
            nc.vector.tensor_copy(out=ms_sb[:], in_=pms[:])
            # broadcast ms over the 48 rows and add the causal mask
            pbc = pp.tile([R, N], F32, tag="big", name=f"pbc{rep}")
            nc.tensor.matmul(
                out=pbc[:], lhsT=one1[:], rhs=ms_sb[:], start=True, stop=True
            )
            nc.vector.tensor_tensor(
                out=mskms[:], in0=pbc[:], in1=maskb_sb[:], op=OP.add
            )

        # ---------- character head (independent of the pair grid; runs
        # before/under the main loop so its PSUM frees early) ----------
        cev = cp.tile([R, 1], F32, name=f"cev{rep}")
        with (
            tc.tile_pool(name=f"ch_sb{rep}", bufs=1) as chp,
            tc.tile_pool(name=f"ch_ps{rep}", bufs=2, space="PSUM") as chps,
        ):
            c1 = chp.tile([128, 3, R], BF16, name=f"c1{rep}")
            for co in range(3):
                pc = chps.tile([128, R], F32, tag="pc", name=f"pc{rep}_{co}")
                for ci in range(HC):
                    nc.tensor.matmul(
                        out=pc[:],
                        lhsT=wc1T_sb[:, ci, co * 128 : (co + 1) * 128],
                        rhs=repsTl_sb[:, ci, :],
                        start=(ci == 0),
                        stop=(ci == HC - 1),
                    )
                nc.scalar.activation(
                    out=c1[:, co, :], in_=pc[:], func=AF.Relu,
                    bias=bc1c_sb[:, co : co + 1],
                )
            plg = chps.tile([R, 18], F32, tag="lg", name=f"plg{rep}")
            for co in range(3):
                nc.tensor.matmul(
                    out=plg[:], lhsT=c1[:, co, :], rhs=wc2T_sb[:, co, :],
                    start=(co == 0), stop=False,
                )
            nc.tensor.matmul(
                out=plg[:], lhsT=one1[:], rhs=bc2r_sb[:], start=False, stop=True
            )
            cm = chp.tile([R, 1], F32, name=f"cm{rep}")
            nc.vector.tensor_reduce(
                out=cm[:], in_=plg[:], axis=mybir.AxisListType.X, op=OP.max
            )
            ncm = chp.tile([R, 1], F32, name=f"ncm{rep}")
            nc.vector.tensor_scalar_mul(ncm[:], cm[:], -1.0)
            cexp = chp.tile([R, 18], F32, name=f"cexp{rep}")
            cz = chp.tile([R, 1], F32, name=f"cz{rep}")
            nc.scalar.activation(
                out=cexp[:], in_=plg[:], func=AF.Exp, bias=ncm[:, 0:1],
                accum_out=cz[:],
            )
            cscr = chp.tile([R, 18], F32, name=f"cscr{rep}")
            nc.vector.tensor_tensor(
                out=cscr[:], in0=plg[:], in1=oneh_sb[:], op=OP.mult
            )
            sl = chp.tile([R, 1], F32, name=f"sl{rep}")
            nc.vector.tensor_reduce(
                out=sl[:], in_=cscr[:], axis=mybir.AxisListType.X, op=OP.add
            )
            lcz = chp.tile([R, 1], F32, name=f"lcz{rep}")
            nc.scalar.activation(out=lcz[:], in_=cz[:], func=AF.Ln)
            nc.vector.tensor_tensor(
                out=cev[:], in0=lcz[:], in1=cm[:], op=OP.add
            )
            nc.vector.tensor_tensor(
                out=cev[:], in0=cev[:], in1=sl[:], op=OP.subtract
            )

        # ---------- main loop: 48 rows of the pair grid ----------
        with (
            tc.tile_pool(name=f"lp_sb{rep}", bufs=2) as lsb,
            tc.tile_pool(name=f"lp_ps{rep}", bufs=2, space="PSUM") as lps,
            tc.tile_pool(name=f"sr_ps{rep}", bufs=2, space="PSUM") as sps,
        ):
            for r in range(R):
                # slot r holds global row i = 8r+1+d on core d (row 0 on
                # core 7's last slot); columns j < i fit in 8(r+1) exactly.
                cols = min(8 * (r + 1), N)
                h1 = lsb.tile(
                    [128, HC, N], F8, tag="h1", name=f"h1_{rep}_{r}",
                    bufs=opts["h1_bufs"],
                )
                for c in range(HC):
                    eng = nc.gpsimd if c >= HC - opts["h1_gp"] else nc.vector
                    eng.tensor_scalar(
                        out=h1[:, c, :cols],
                        in0=at_sb[:, c, :cols],
                        scalar1=bb_sb[:, c, r : r + 1],
                        scalar2=0.0,
                        op0=OP.add,
                        op1=OP.max,
                    )
                h2t = lsb.tile(
                    [128, 3, N], F8, tag="h2t", name=f"h2t_{rep}_{r}",
                    bufs=opts["h2t_bufs"],
                )
                ph = lps.tile(
                    [128, 3, 512], F32, tag="ph", name=f"ph{rep}_{r}", bufs=2
                )
                # DoubleRow disables fast-weight-load and is a net loss
                # below 128 free columns; small slots use regular fp8 MMs.
                use_dr = cols >= 128
                for hb in range(3):
                    if use_dr:
                        for cc in range(0, HC, 2):
                            nc.tensor.matmul(
                                out=ph[:, hb, :cols],
                                lhsT=w28_sb[
                                    :, cc : cc + 2, hb * 128 : (hb + 1) * 128
                                ],
                                rhs=h1[:, cc : cc + 2, :cols],
                                start=(cc == 0),
                                stop=(cc == HC - 2),
                                perf_mode=DR,
                            )
                    else:
                        for cc in range(HC):
                            nc.tensor.matmul(
                                out=ph[:, hb, :cols],
                                lhsT=w28_sb[:, cc, hb * 128 : (hb + 1) * 128],
                                rhs=h1[:, cc, :cols],
                                start=(cc == 0),
                                stop=(cc == HC - 1),
                            )
                if fuse_relu:
                    # one 3D-AP relu over all 3 output blocks (b_pair2 == 0)
                    nc.scalar.activation(
                        out=h2t[:, :, :cols],
                        in_=ph[:, :, :cols],
                        func=AF.Relu,
                        scale=1.0 / S2,
                    )
                else:
                    for hb in range(3):
                        nc.scalar.activation(
                            out=h2t[:, hb, :cols],
                            in_=ph[:, hb, :cols],
                            func=AF.Relu,
                            bias=b2c_sb[:, hb : hb + 1],
                            scale=1.0 / S2,
                        )
                sr = sps.tile([1, N], F32, tag="srow", name=f"sr{rep}_{r}")
                if use_dr:
                    nc.tensor.matmul(
                        out=sr[:, :cols],
                        lhsT=w38_sb[:, 0:2, 0:1],
                        rhs=h2t[:, 0:2, :cols],
                        start=True,
                        stop=False,
                        perf_mode=DR,
                    )
                    nc.tensor.matmul(
                        out=sr[:, :cols],
                        lhsT=w38_sb[:, 2, 0:1],
                        rhs=h2t[:, 2, :cols],
                        start=False,
                        stop=True,
                    )
                else:
                    for c in range(3):
                        nc.tensor.matmul(
                            out=sr[:, :cols],
                            lhsT=w38_sb[:, c, 0:1],
                            rhs=h2t[:, c, :cols],
                            start=(c == 0),
                            stop=(c == 2),
                        )
                # scores live on partition 0; engines can't shift
                # partitions, so stage flat and DMA-reshape later. The
                # 1/S3 fp8 descale folds into the copy; rotate engines
                # to split the load.
                # GPSIMD cannot read PSUM on HW; split DVE/ACT.
                dst = sblkf[:, r, :cols]
                cm = opts["copy_mode"]
                use_dve = (r % 2 == 0) if cm == "alt" else (cm == "dve")
                if use_dve:
                    nc.vector.tensor_scalar_mul(dst, sr[:, :cols], 1.0 / S3)
                else:
                    nc.scalar.mul(out=dst, in_=sr[:, :cols], mul=1.0 / S3)

        # ---------- epilogue: masked row-softmax loss + char CE ----------
        with (
            tc.tile_pool(name=f"ep_sb{rep}", bufs=1) as ep,
            tc.tile_pool(name=f"ep_ps{rep}", bufs=2, space="PSUM") as eps,
        ):
            # reshape valid score prefixes to [R, N] (per-slot DMAs ride
            # the idle queue); sblk was pre-zeroed so masked tails stay 0.
            for r in range(R):
                cols = min(8 * (r + 1), N)
                nc.sync.dma_start(
                    out=sblk[r : r + 1, :cols], in_=sblkf[:, r, :cols]
                )
            # row-softmax in 16-row groups so each group's chain starts as
            # soon as its score rows land (overlaps the main loop's tail).
            # Full-height tiles + sliced ops keep partitions lane-aligned.
            x = ep.tile([R, N], F32, name=f"x{rep}")
            rm = ep.tile([R, 1], F32, name=f"rm{rep}")
            nrm = ep.tile([R, 1], F32, name=f"nrm{rep}")
            pexp = ep.tile([R, N], F32, name=f"pexp{rep}")
            z = ep.tile([R, 1], F32, name=f"z{rep}")
            escr = ep.tile([R, N], F32, name=f"escr{rep}")
            e = ep.tile([R, 1], F32, name=f"e{rep}")
            lz = ep.tile([R, 1], F32, name=f"lz{rep}")
            le = ep.tile([R, 1], F32, name=f"le{rep}")
            tnll = ep.tile([R, 1], F32, name=f"tnll{rep}")
            for g0, g1 in ((0, 32), (32, 48)):  # engine partition base
                gs = slice(g0, g1)              # must be 32-aligned
                nc.vector.tensor_tensor(
                    out=x[gs, :], in0=sblk[gs, :], in1=mskms[gs, :], op=OP.add
                )
                nc.vector.tensor_reduce(
                    out=rm[gs, :], in_=x[gs, :], axis=mybir.AxisListType.X,
                    op=OP.max,
                )
                nc.vector.tensor_scalar_mul(nrm[gs, :], rm[gs, :], -1.0)
                nc.scalar.activation(
                    out=pexp[gs, :], in_=x[gs, :], func=AF.Exp,
                    bias=nrm[gs, 0:1], accum_out=z[gs, :],
                )
                nc.vector.tensor_tensor(
                    out=escr[gs, :], in0=pexp[gs, :], in1=multb_sb[gs, :],
                    op=OP.mult,
                )
                nc.vector.tensor_reduce(
                    out=e[gs, :], in_=escr[gs, :], axis=mybir.AxisListType.X,
                    op=OP.add,
                )
                nc.scalar.activation(out=lz[gs, :], in_=z[gs, :], func=AF.Ln)
                nc.scalar.activation(out=le[gs, :], in_=e[gs, :], func=AF.Ln)
                nc.vector.tensor_tensor(
                    out=tnll[gs, :], in0=lz[gs, :], in1=le[gs, :],
                    op=OP.subtract,
                )
            pl = eps.tile([1, 1], F32, tag="loss", name=f"pl{rep}", bufs=1)
            nc.tensor.matmul(
                out=pl[:], lhsT=tnll[:, 0:1], rhs=wnll_sb[:], start=True,
                stop=False,
            )
            nc.tensor.matmul(
                out=pl[:], lhsT=cev[:, 0:1], rhs=wch_sb[:], start=False,
                stop=True,
            )
            lout = ep.tile([1, 1], F32, name=f"lout{rep}")
            nc.vector.tensor_copy(out=lout[:], in_=pl[:])
            nc.sync.dma_start(out=io["loss"].ap(), in_=lout[:])


def _chunk_cols(w):
    """[K, O] -> [128, K//128, O]  (partition-chunked contraction dim)."""
    k, o = w.shape
    return np.ascontiguousarray(w.reshape(k // 128, 128, o).transpose(1, 0, 2))


def _chunk_vec(v, ncol):
    """[C] -> [128, ncol] column-chunks (zero padded)."""
    out = np.zeros((128, ncol), np.float32)
    for c in range(ncol):
        seg = v[c * 128 : (c + 1) * 128]
        out[: len(seg), c] = seg
    return out


def _core_rows(d):
    """Slot->global-row map for core d (16 rows of each chunk class)."""
    c1 = list(range(1, 129))
    c2 = list(range(129, 257))
    c3 = list(range(257, 384)) + [0]
    return c1[d::NC_] + c2[d::NC_] + c3[d::NC_]


def _prep_in_maps(inputs):
    bf = ml_dtypes.bfloat16
    f8 = ml_dtypes.float8_e4m3

    seq = np.asarray(inputs["sequence_output"], np.float32)
    spk = np.asarray(inputs["speaker_emb"], np.float32)
    dummy = np.asarray(inputs["dummy_emb"], np.float32)
    seg = np.asarray(inputs["mentions_seg"]).astype(np.int64)
    mstart = np.asarray(inputs["mention_start"]).astype(np.int64)
    mend = np.asarray(inputs["mention_end"]).astype(np.int64)
    sid = np.asarray(inputs["speaker_ids"]).astype(np.int64)[seg, mstart]
    reps = seq[seg, mstart] + seq[seg, mend] + spk[sid]
    all_reps = np.concatenate([dummy, reps], axis=0)          # [N, H]

    def chunkT(a):
        # [n, H] -> [128, HC, n] transposed layout
        n = a.shape[0]
        return np.ascontiguousarray(
            a.T.reshape(HC, 128, n).transpose(1, 0, 2)
        )

    reps8 = chunkT(all_reps).astype(f8)

    W_pair1 = np.asarray(inputs["W_pair1"], np.float32)
    wa8 = _chunk_cols(np.ascontiguousarray(W_pair1[:, :H].T) * S2).astype(f8)
    wb8 = _chunk_cols(np.ascontiguousarray(W_pair1[:, H:].T) * S2).astype(f8)
    w28 = _chunk_cols(
        np.ascontiguousarray(np.asarray(inputs["W_pair2"], np.float32).T) * S2
    ).astype(f8)
    w38 = np.zeros((128, 3, 16), np.float32)
    w38[:, :, 0] = _chunk_vec(
        np.asarray(inputs["W_pair3"], np.float32)[0] * S3, 3
    )
    w38 = w38.astype(f8)
    b1c = _chunk_vec(np.asarray(inputs["b_pair1"], np.float32), HC)
    b2c = _chunk_vec(np.asarray(inputs["b_pair2"], np.float32), 3)
    wm18 = _chunk_cols(
        np.ascontiguousarray(np.asarray(inputs["W_m1"], np.float32).T) * S2
    ).astype(f8)
    bm1c = _chunk_vec(np.asarray(inputs["b_m1"], np.float32), 3)
    wm2T = _chunk_cols(
        np.ascontiguousarray(np.asarray(inputs["W_m2"], np.float32).T)
    ).astype(bf)
    bm2c = _chunk_vec(np.asarray(inputs["b_m2"], np.float32), 2)
    wm3c = _chunk_vec(np.asarray(inputs["W_m3"], np.float32)[0], 2).astype(bf)
    wc1T = _chunk_cols(
        np.ascontiguousarray(np.asarray(inputs["W_c1"], np.float32).T)
    ).astype(bf)
    bc1c = _chunk_vec(np.asarray(inputs["b_c1"], np.float32), 3)
    wc2T = _chunk_cols(
        np.ascontiguousarray(np.asarray(inputs["W_c2"], np.float32).T)
    ).astype(bf)
    bc2r = np.asarray(inputs["b_c2"], np.float32).reshape(1, 18)

    link_first = np.asarray(inputs["link_first"]).astype(np.int64)
    link_second = np.asarray(inputs["link_second"]).astype(np.int64)
    label = np.asarray(inputs["character_label"]).astype(np.int64)

    mult = np.zeros((N, N), np.float32)
    np.add.at(mult, (link_second, link_first), 1.0)
    has_link = mult.sum(axis=1) > 0
    wnll_full = ((np.arange(N) >= 1) & has_link).astype(np.float32)
    mult[~has_link, 0] = 1.0  # keep log(E) finite; weight is 0 there

    mask_full = np.where(
        np.arange(N)[None, :] >= np.arange(N)[:, None], np.float32(NEG), 0.0
    ).astype(np.float32)

    oneh_full = np.zeros((N, 18), np.float32)
    wch_full = np.zeros(N, np.float32)
    oneh_full[np.arange(1, N), label] = 1.0
    wch_full[1:] = 1.0

    shared = dict(
        reps8=reps8,
        wa8=wa8, wb8=wb8, w28=w28, w38=w38, b1c=b1c, b2c=b2c,
        wm18=wm18, bm1c=bm1c, wm2T=wm2T, bm2c=bm2c, wm3c=wm3c,
        wc1T=wc1T, bc1c=bc1c, wc2T=wc2T, bc2r=bc2r,
        zrow=np.zeros((R, N), np.float32),
    )
    in_maps = []
    for d in range(NC_):
        rows = _core_rows(d)
        m = dict(shared)
        m["repsTl"] = chunkT(all_reps[rows]).astype(bf)
        m["repsTl8"] = chunkT(all_reps[rows]).astype(f8)
        m["maskb"] = np.ascontiguousarray(mask_full[rows])
        m["multb"] = np.ascontiguousarray(mult[rows])
        m["wnll"] = np.ascontiguousarray(wnll_full[rows]).reshape(R, 1)
        m["oneh"] = np.ascontiguousarray(oneh_full[rows])
        m["wch"] = np.ascontiguousarray(wch_full[rows]).reshape(R, 1)
        in_maps.append(m)
    return in_maps


def kernel(**inputs):
    global LAST_RESULT
    in_maps = _prep_in_maps(inputs)

    # the fused h2 relu drops the (per-spec zero) b_pair2 bias
    fuse = not np.any(np.asarray(inputs["b_pair2"], np.float32))
    key = ("nc", fuse)
    if key not in _CACHE:
        _CACHE[key] = _build_program(fuse_relu=fuse)
    nc = _CACHE[key]

    res = run_bass_kernel_spmd(nc, in_maps, core_ids=list(range(NC_)))
    LAST_RESULT = res
    total = np.float32(0.0)
    for d in range(NC_):
        total += np.float32(res.results[d]["loss"][0, 0])
    return np.asarray(total, dtype=np.float32)


if __name__ == "__main__":
    import reference

    inputs = {k: np.asarray(v) for k, v in reference.setup_inputs().items()}
    out = kernel(**inputs)
    print("kernel out:", out)



# revision 21
# speedup vs baseline: 1935.5632x; 1935.5632x over previous
"""Trainium2 Bass kernel for nn_JointLearningModel (coref-style joint model).

Sharding: the 384x384 pair grid is split by rows across 8 NeuronCores.
Mention representations are computed on the host (pure gathers) and
uploaded replicated in transposed [H, N] layout; params replicated; the
scalar loss is computed per-core over its row block (+ its slice of the
character CE) and summed on the host.

v2 structure (vs the per-row baseline):
- Rows are bin-packed into ~18 blocks of whole rows totalling <=512
  pairs each. The pair-MLP (h1 -> W2 -> relu -> W3) runs per BLOCK, so
  the dominant W_pair2 fp8 DoubleRow matmuls stream 400-512 free
  columns and reload weights ~18x instead of ~780x (DR disables fast
  weight load, so per-row reloads dominated the old kernel on HW).
- Score rows stay S3-scaled in PSUM and are DMA'd straight into the
  per-row [R, N] score block; the 1/S3 fp8 descale folds into the
  epilogue softmax (exp scale and rowmax negate), removing all per-row
  ACT copies and the second reshape-DMA pass.
- Every core gets the identical cols multiset {8,16,...,384}, so the
  SPMD instruction stream is identical across cores.
"""

import numpy as np
import ml_dtypes

import concourse.bass as bass
import concourse.mybir as mybir
import concourse.tile as tile
from concourse import bacc
from concourse.bass_utils import run_bass_kernel_spmd

F32 = mybir.dt.float32
BF16 = mybir.dt.bfloat16
F8 = mybir.dt.float8e4
I32 = mybir.dt.int32
AF = mybir.ActivationFunctionType
OP = mybir.AluOpType
DR = mybir.MatmulPerfMode.DoubleRow

B, L, H, M = 8, 512, 768, 383
N = M + 1          # 384 rows/cols of the pair grid
NC_ = 8            # cores
R = N // NC_       # 48 rows per core
HC = H // 128      # 6 k-chunks of the hidden dim
NEG = -10000.0
S2 = 16.0          # fp8 pre-scale on W_pair2
S3 = 16.0          # fp8 pre-scale on W_pair3 (descale folded into softmax)
PB = 512           # max pairs per block (PSUM: [128,3,512] f32 = 3 banks)

_CACHE = {}
LAST_RESULT = None


def _pack_blocks():
    """Bin-pack the per-core cols multiset {8,16,...,384} into blocks of
    whole rows totalling <= PB pairs. Returns a list of blocks; each
    block is a list of (slot, off, cols): `slot` is the epilogue row
    index (assigned in block order so early blocks fill the first
    softmax group), `off` the pair offset inside the block."""
    ks = list(range(R, 1, -1))          # cols/8, descending (FFD)
    bins = []
    for k in ks:
        for b in bins:
            if sum(b) + k <= PB // 8:
                b.append(k)
                break
        else:
            bins.append([k])
    # many-seg bins first (their scatters spread out early), the lone
    # 8-pair row last so the post-loop relu->W3->scatter tail is tiny
    bins.sort(key=lambda b: (-len(b), -sum(b)))
    bins.append([1])
    blocks, slot = [], 0
    for b in bins:
        segs, off = [], 0
        for k in b:
            segs.append((slot, off, 8 * k))
            slot += 1
            off += 8 * k
        blocks.append(segs)
    assert slot == R
    assert sum(ln for bl in blocks for _, _, ln in bl) == 8 * (R * (R + 1) // 2)
    return blocks


# slot -> cols map (same for all cores; derived from the packing)
_BLOCKS = _pack_blocks()
_SLOT_COLS = [0] * R
for _bl in _BLOCKS:
    for _s, _o, _ln in _bl:
        _SLOT_COLS[_s] = _ln


def _build_program(reps=1, fuse_relu=True, gp_chunks=1, h1_bufs=6, h2t_bufs=3):
    nc = bacc.Bacc(
        "TRN2", target_bir_lowering=False, debug=False, enable_asserts=False
    )

    def din(name, shape, dt):
        return nc.dram_tensor(name, list(shape), dt, kind="ExternalInput")

    # mention representations (host-gathered), transposed layouts
    reps8 = din("reps8", [128, HC, N], F8)      # reps8[p,c,j] = reps[j, 128c+p]
    repsTl8 = din("repsTl8", [128, HC, R], F8)  # local rows, slot order
    # pair MLP weights (fp8, pre-scaled by S2)
    wa8 = din("wa8", [128, HC, H], F8)          # wa8[p,ci,o] = Wa[o, 128ci+p]*S2
    wb8 = din("wb8", [128, HC, H], F8)
    w28 = din("w28", [128, HC, H // 2], F8)     # W2.T * S2, fp8
    # wm18p packs W_m1.T*S2 in [:, :, :384] and W_pair3*S3 at
    # [:, c, 384] (chunk stride 400 keeps DoubleRow 16B-aligned)
    wm18p = din("wm18p", [128, HC, 400], F8)
    wm2T = din("wm2T", [128, 3, H // 4], BF16)
    wm3c = din("wm3c", [128, 2], BF16)
    wc1T = din("wc1T", [128, HC, H // 2], BF16)
    wc2T = din("wc2T", [128, 3, 18], BF16)
    # packed small biases: b1c|b2c|bm1c|bm2c|bc1c along the free dim
    pk128 = din("pk128", [128, 17], F32)
    # packed per-row plumbing (slot order, maskb pre-scaled by S3):
    # maskb|multb|oneh|wnll|wch, bc2r at [0:1, 788:806]
    pk48 = din("pk48", [R, 806], F32)

    loss = nc.dram_tensor("loss", [1, 1], F32, kind="ExternalOutput")

    opts = dict(fuse_relu=fuse_relu, gp_chunks=gp_chunks, h1_bufs=h1_bufs,
                h2t_bufs=h2t_bufs)
    with tile.TileContext(nc) as tc:
        for rep in range(reps):
            _build_rep(nc, tc, rep, opts, dict(
                reps8=reps8, repsTl8=repsTl8, wa8=wa8, wb8=wb8, w28=w28,
                wm18p=wm18p, wm2T=wm2T, wm3c=wm3c, wc1T=wc1T, wc2T=wc2T,
                pk128=pk128, pk48=pk48, loss=loss,
            ))

    nc.compile()
    return nc


def _build_rep(nc, tc, rep, opts, io):
    fuse_relu = opts["fuse_relu"]
    with tc.tile_pool(name=f"const{rep}", bufs=1) as cp:
        def load(name, h, q=None):
            t = cp.tile(list(h.shape), h.dtype, name=f"{name}{rep}")
            (q or nc.sync).dma_start(out=t[:], in_=h.ap())
            return t

        # split the loads over both HWDGE queues (SP + ACT) so the Bb
        # chain (wb8/repsTl8/pk128, scalar q) starts in parallel with
        # the A.T chain (reps8/wa8, sync q); late-use loads go last
        reps8_sb = load("reps8_sb", io["reps8"])
        wa8_sb = load("wa8_sb", io["wa8"])
        wb8_sb = load("wb8_sb", io["wb8"], q=nc.scalar)
        repsTl8_sb = load("repsTl8_sb", io["repsTl8"], q=nc.scalar)
        pk128_sb = load("pk128_sb", io["pk128"], q=nc.scalar)
        w28_sb = load("w28_sb", io["w28"])
        wm18p_sb = load("wm18p_sb", io["wm18p"])
        wm2T_sb = load("wm2T_sb", io["wm2T"])
        wm3c_sb = load("wm3c_sb", io["wm3c"])
        wc2T_sb = load("wc2T_sb", io["wc2T"])
        pk48_sb = load("pk48_sb", io["pk48"], q=nc.scalar)
        wc1T_sb = load("wc1T_sb", io["wc1T"], q=nc.scalar)

        one1 = cp.tile([1, R], F32, name=f"one1{rep}")
        nc.vector.memset(one1[:], 1.0)

        # outputs of the preamble, used by the main loop / epilogue
        at_sb = cp.tile([128, HC, N], BF16, name=f"at_sb{rep}")   # A.T
        bb_sb = cp.tile([128, HC, R], F32, name=f"bb_sb{rep}")    # Bm.T + b1
        mskms = cp.tile([R, N], F32, name=f"mskms{rep}")          # S3*(mask+ms)
        # Scores land here (S3-scaled), row per slot. Pre-zeroed so
        # rows' uncomputed masked tails read as 0.
        sblk = cp.tile([R, N], F32, name=f"sblk{rep}")
        nc.gpsimd.memset(sblk[:], 0.0)

        # ---------- preamble matmuls: A.T, Bb, ms, mask+ms ----------
        # A.T and Bb are emitted chunk-interleaved so chunk 0 of both is
        # ready early and the first main-loop h1 op can start sooner.
        # Bb first (its wb8/repsTl8 loads land early on the scalar
        # queue and the DVE bb copies gate the FIFO'd h1 stream), then
        # A.T chunk by chunk so h1 chunk c can start as at_c lands.
        with tc.tile_pool(name=f"pre_ps{rep}", bufs=2, space="PSUM") as pp:
            for co in range(HC):
                pb_ = pp.tile([128, R], F32, tag="small", name=f"pb{rep}_{co}")
                for ci in range(0, HC, 2):
                    nc.tensor.matmul(
                        out=pb_[:],
                        lhsT=wb8_sb[:, ci : ci + 2, co * 128 : (co + 1) * 128],
                        rhs=repsTl8_sb[:, ci : ci + 2, :],
                        start=(ci == 0),
                        stop=(ci == HC - 2),
                        perf_mode=DR,
                    )
                nc.vector.tensor_scalar(
                    out=bb_sb[:, co, :],
                    in0=pb_[:],
                    scalar1=1.0 / S2,
                    scalar2=pk128_sb[:, co : co + 1],
                    op0=OP.mult,
                    op1=OP.add,
                )
            for co in range(HC):
                pa = pp.tile([128, N], F32, tag="big", name=f"pa{rep}_{co}")
                for ci in range(0, HC, 2):
                    nc.tensor.matmul(
                        out=pa[:],
                        lhsT=wa8_sb[:, ci : ci + 2, co * 128 : (co + 1) * 128],
                        rhs=reps8_sb[:, ci : ci + 2, :],
                        start=(ci == 0),
                        stop=(ci == HC - 2),
                        perf_mode=DR,
                    )
                nc.scalar.mul(out=at_sb[:, co, :], in_=pa[:], mul=1.0 / S2)
        # SBUF homes for the ms-MLP / char-head stages that are
        # interleaved into the main loop (their PSUM tiles rotate
        # through the score-row tag, written+consumed within a stage)
        ms1 = cp.tile([128, 3, N], BF16, name=f"ms1{rep}")
        ms2 = cp.tile([128, 2, N], BF16, name=f"ms2{rep}")
        ms_sb = cp.tile([1, N], F32, name=f"ms_sb{rep}")
        c1 = cp.tile([128, 3, R], BF16, name=f"c1{rep}")
        cev = cp.tile([R, 1], F32, name=f"cev{rep}")
        cm = cp.tile([R, 1], F32, name=f"cm{rep}")
        ncm = cp.tile([R, 1], F32, name=f"ncm{rep}")
        cexp = cp.tile([R, 18], F32, name=f"cexp{rep}")
        cz = cp.tile([R, 1], F32, name=f"cz{rep}")
        cscr = cp.tile([R, 18], F32, name=f"cscr{rep}")
        sl = cp.tile([R, 1], F32, name=f"sl{rep}")
        lcz = cp.tile([R, 1], F32, name=f"lcz{rep}")

        # ---------- main loop: pair blocks (whole rows, <=PB pairs) ----------
        # Engine queues are strict FIFO with no bypass, so a queued op
        # waiting on a same-block cross-engine result (W3 on relu, srf
        # copy on W3) head-of-line-blocks the NEXT block's work on that
        # engine. Software-pipeline the emission: W3 runs one block
        # behind W2/relu, the srf copy + scatter DMAs two behind.
        gp_lo = HC - opts["gp_chunks"]
        nb = len(_BLOCKS)
        blk_pb = [segs[-1][1] + segs[-1][2] for segs in _BLOCKS]
        h2t_t, sr_t = [None] * nb, [None] * nb
        with (
            tc.tile_pool(name=f"lp_sb{rep}", bufs=2) as lsb,
            tc.tile_pool(name=f"lp_ps{rep}", bufs=2, space="PSUM") as lps,
            tc.tile_pool(name=f"sr_ps{rep}", bufs=2, space="PSUM") as sps,
        ):
            def emit_front(bi):          # h1 -> W2 -> relu for block bi
                segs, pb = _BLOCKS[bi], blk_pb[bi]
                h1 = lsb.tile(
                    [128, HC, PB], F8, tag="h1", name=f"h1_{rep}_{bi}",
                    bufs=opts["h1_bufs"],
                )
                for c in range(HC):
                    eng = nc.gpsimd if c >= gp_lo else nc.vector
                    for slot, off, ln in segs:
                        eng.tensor_scalar(
                            out=h1[:, c, off : off + ln],
                            in0=at_sb[:, c, :ln],
                            scalar1=bb_sb[:, c, slot : slot + 1],
                            scalar2=0.0,
                            op0=OP.add,
                            op1=OP.max,
                        )
                ph = lps.tile(
                    [128, 3, PB], F32, tag="ph", name=f"ph{rep}_{bi}", bufs=2
                )
                for hb in range(3):
                    for cc in range(0, HC, 2):
                        nc.tensor.matmul(
                            out=ph[:, hb, :pb],
                            lhsT=w28_sb[
                                :, cc : cc + 2, hb * 128 : (hb + 1) * 128
                            ],
                            rhs=h1[:, cc : cc + 2, :pb],
                            start=(cc == 0),
                            stop=(cc == HC - 2),
                            perf_mode=DR,
                        )
                h2t = lsb.tile(
                    [128, 3, PB], F8, tag="h2t", name=f"h2t_{rep}_{bi}",
                    bufs=opts["h2t_bufs"],
                )
                if fuse_relu:
                    # one 3D-AP relu over all 3 output blocks (b_pair2 == 0)
                    nc.scalar.activation(
                        out=h2t[:, :, :pb],
                        in_=ph[:, :, :pb],
                        func=AF.Relu,
                        scale=1.0 / S2,
                    )
                else:
                    for hb in range(3):
                        nc.scalar.activation(
                            out=h2t[:, hb, :pb],
                            in_=ph[:, hb, :pb],
                            func=AF.Relu,
                            bias=pk128_sb[:, 6 + hb : 7 + hb],
                            scale=1.0 / S2,
                        )
                h2t_t[bi] = h2t

            def emit_w3(bi):             # W3 score row for block bi
                pb = blk_pb[bi]
                sr = sps.tile([1, PB], F32, tag="srow", name=f"sr{rep}_{bi}")
                nc.tensor.matmul(
                    out=sr[:, :pb],
                    lhsT=wm18p_sb[:, 0:2, 384:385],
                    rhs=h2t_t[bi][:, 0:2, :pb],
                    start=True,
                    stop=False,
                    perf_mode=DR,
                )
                nc.tensor.matmul(
                    out=sr[:, :pb],
                    lhsT=wm18p_sb[:, 2, 384:385],
                    rhs=h2t_t[bi][:, 2, :pb],
                    start=False,
                    stop=True,
                )
                sr_t[bi] = sr

            def emit_scatter(bi):        # PSUM->SBUF stage + row DMAs
                segs, pb = _BLOCKS[bi], blk_pb[bi]
                srf = lsb.tile(
                    [1, PB], F32, tag="srf", name=f"srf{rep}_{bi}", bufs=3
                )
                if bi >= nb - 3:
                    nc.vector.tensor_scalar_mul(srf[:, :pb], sr_t[bi][:, :pb], 1.0)
                else:
                    nc.scalar.mul(out=srf[:, :pb], in_=sr_t[bi][:, :pb], mul=1.0)
                for si, (slot, off, ln) in enumerate(segs):
                    q = (nc.sync, nc.scalar, nc.gpsimd)[(bi + si) % 3]
                    q.dma_start(
                        out=sblk[slot : slot + 1, :ln],
                        in_=srf[:, off : off + ln],
                    )

            # ms-MLP / char-head stages, one per other block iteration.
            # Each stage writes its PSUM tile and consumes it in the
            # same stage so the shared "srow" tag rotation never makes
            # a later score row wait on a not-yet-run reader.
            def s_ms1(i):
                pm = sps.tile([128, N], F32, tag="srow", name=f"pm{rep}_{i}")
                for ci in range(0, HC, 2):
                    nc.tensor.matmul(
                        out=pm[:],
                        lhsT=wm18p_sb[:, ci : ci + 2, i * 128 : (i + 1) * 128],
                        rhs=reps8_sb[:, ci : ci + 2, :],
                        start=(ci == 0),
                        stop=(ci == HC - 2),
                        perf_mode=DR,
                    )
                nc.scalar.activation(
                    out=ms1[:, i, :], in_=pm[:], func=AF.Relu,
                    bias=pk128_sb[:, 9 + i : 10 + i], scale=1.0 / S2,
                )

            def s_ms2(i):
                sz = (128, 64)[i]
                pm2 = sps.tile([128, N], F32, tag="srow", name=f"pm2{rep}_{i}")
                for ci in range(3):
                    nc.tensor.matmul(
                        out=pm2[:sz, :],
                        lhsT=wm2T_sb[:, ci, i * 128 : i * 128 + sz],
                        rhs=ms1[:, ci, :],
                        start=(ci == 0),
                        stop=(ci == 2),
                    )
                nc.scalar.activation(
                    out=ms2[:sz, i, :], in_=pm2[:sz, :], func=AF.Relu,
                    bias=pk128_sb[:sz, 12 + i : 13 + i],
                )

            def s_pms():
                pms = sps.tile([1, N], F32, tag="srow", name=f"pms{rep}")
                nc.tensor.matmul(
                    out=pms[:], lhsT=wm3c_sb[:, 0:1], rhs=ms2[:, 0, :],
                    start=True, stop=False,
                )
                nc.tensor.matmul(
                    out=pms[:], lhsT=wm3c_sb[:64, 1:2], rhs=ms2[:64, 1, :],
                    start=False, stop=True,
                )
                # ms is S3-scaled; maskb comes pre-scaled from the host
                nc.vector.tensor_scalar_mul(ms_sb[:], pms[:], S3)

            def s_pbc():
                pbc = sps.tile([R, N], F32, tag="srow", name=f"pbc{rep}")
                nc.tensor.matmul(
                    out=pbc[:], lhsT=one1[:], rhs=ms_sb[:],
                    start=True, stop=True,
                )
                nc.vector.tensor_tensor(
                    out=mskms[:], in0=pbc[:], in1=pk48_sb[:, 0:384], op=OP.add
                )

            def s_pc(i):
                pc = sps.tile([128, R], F32, tag="srow", name=f"pc{rep}_{i}")
                for ci in range(HC):
                    nc.tensor.matmul(
                        out=pc[:],
                        lhsT=wc1T_sb[:, ci, i * 128 : (i + 1) * 128],
                        rhs=repsTl8_sb[:, ci, :],
                        start=(ci == 0),
                        stop=(ci == HC - 1),
                    )
                nc.scalar.activation(
                    out=c1[:, i, :], in_=pc[:], func=AF.Relu,
                    bias=pk128_sb[:, 14 + i : 15 + i],
                )

            def s_plg():
                plg = sps.tile([R, 18], F32, tag="srow", name=f"plg{rep}")
                for co in range(3):
                    nc.tensor.matmul(
                        out=plg[:], lhsT=c1[:, co, :], rhs=wc2T_sb[:, co, :],
                        start=(co == 0), stop=False,
                    )
                nc.tensor.matmul(
                    out=plg[:], lhsT=one1[:], rhs=pk48_sb[0:1, 788:806],
                    start=False, stop=True,
                )
                nc.vector.tensor_reduce(
                    out=cm[:], in_=plg[:], axis=mybir.AxisListType.X, op=OP.max
                )
                nc.vector.tensor_scalar_mul(ncm[:], cm[:], -1.0)
                nc.scalar.activation(
                    out=cexp[:], in_=plg[:], func=AF.Exp, bias=ncm[:, 0:1],
                    accum_out=cz[:],
                )
                nc.vector.tensor_tensor(
                    out=cscr[:], in0=plg[:], in1=pk48_sb[:, 768:786], op=OP.mult
                )
                nc.vector.tensor_reduce(
                    out=sl[:], in_=cscr[:], axis=mybir.AxisListType.X, op=OP.add
                )

            def s_cev():
                nc.scalar.activation(out=lcz[:], in_=cz[:], func=AF.Ln)
                nc.vector.tensor_tensor(
                    out=cev[:], in0=lcz[:], in1=cm[:], op=OP.add
                )
                nc.vector.tensor_tensor(
                    out=cev[:], in0=cev[:], in1=sl[:], op=OP.subtract
                )

            # ---- epilogue closures: masked row-softmax + loss dot ----
            # Emitted mid-loop as soon as a group's score rows have
            # scattered; x stays S3-scaled, the exp descales via its
            # scale arg and the rowmax negate uses -1/S3.
            x = cp.tile([R, N], F32, name=f"x{rep}")
            rm = cp.tile([R, 1], F32, name=f"rm{rep}")
            nrm = cp.tile([R, 1], F32, name=f"nrm{rep}")
            pexp = cp.tile([R, N], F32, name=f"pexp{rep}")
            z = cp.tile([R, 1], F32, name=f"z{rep}")
            escr = cp.tile([R, N], F32, name=f"escr{rep}")
            e = cp.tile([R, 1], F32, name=f"e{rep}")
            lz = cp.tile([R, 1], F32, name=f"lz{rep}")
            le = cp.tile([R, 1], F32, name=f"le{rep}")
            tnll = cp.tile([R, 1], F32, name=f"tnll{rep}")

            def emit_softmax(g0, g1):       # partition slices, 32-aligned
                gs = slice(g0, g1)
                nc.vector.tensor_tensor(
                    out=x[gs, :], in0=sblk[gs, :], in1=mskms[gs, :], op=OP.add
                )
                nc.vector.tensor_reduce(
                    out=rm[gs, :], in_=x[gs, :], axis=mybir.AxisListType.X,
                    op=OP.max,
                )
                nc.vector.tensor_scalar_mul(nrm[gs, :], rm[gs, :], -1.0 / S3)
                nc.scalar.activation(
                    out=pexp[gs, :], in_=x[gs, :], func=AF.Exp,
                    bias=nrm[gs, 0:1], scale=1.0 / S3, accum_out=z[gs, :],
                )
                nc.vector.tensor_tensor(
                    out=escr[gs, :], in0=pexp[gs, :], in1=pk48_sb[gs, 384:768],
                    op=OP.mult,
                )
                nc.vector.tensor_reduce(
                    out=e[gs, :], in_=escr[gs, :], axis=mybir.AxisListType.X,
                    op=OP.add,
                )
                nc.scalar.activation(out=lz[gs, :], in_=z[gs, :], func=AF.Ln)
                nc.scalar.activation(out=le[gs, :], in_=e[gs, :], func=AF.Ln)
                nc.vector.tensor_tensor(
                    out=tnll[gs, :], in0=lz[gs, :], in1=le[gs, :],
                    op=OP.subtract,
                )

            def emit_loss():
                pl = sps.tile([1, 16], F32, tag="srow", name=f"pl{rep}")
                nc.tensor.matmul(
                    out=pl[:, 0:1], lhsT=tnll[:, 0:1], rhs=pk48_sb[:, 786:787],
                    start=True, stop=False,
                )
                nc.tensor.matmul(
                    out=pl[:, 0:1], lhsT=cev[:, 0:1], rhs=pk48_sb[:, 787:788],
                    start=False, stop=True,
                )
                lout = cp.tile([1, 1], F32, name=f"lout{rep}")
                nc.vector.tensor_copy(out=lout[:], in_=pl[:, 0:1])
                nc.sync.dma_start(out=io["loss"].ap(), in_=lout[:])

            stages = [
                lambda: s_ms1(0), lambda: s_ms1(1), lambda: s_ms1(2),
                lambda: s_ms2(0), lambda: s_ms2(1), s_pms, s_pbc,
                lambda: s_pc(0), lambda: s_pc(1), lambda: s_pc(2),
                s_plg, s_cev,
            ]
            # earliest iteration whose scatter covers slots 0..31
            nseg_cum, b31 = 0, nb - 1
            for i, segs in enumerate(_BLOCKS):
                nseg_cum += len(segs)
                if nseg_cum >= 32:
                    b31 = i
                    break
            for bi in range(nb + 5):
                if 0 <= bi - 2 < nb:
                    emit_w3(bi - 2)
                if 0 <= bi - 3 < nb:
                    emit_scatter(bi - 3)
                if bi < nb:
                    emit_front(bi)
                if bi < len(stages):
                    stages[bi]()
                if bi == b31 + 4:
                    emit_softmax(0, 32)
                if bi == nb + 3:
                    emit_softmax(32, 48)
                if bi == nb + 4:
                    emit_loss()


def _chunk_cols(w):
    """[K, O] -> [128, K//128, O]  (partition-chunked contraction dim)."""
    k, o = w.shape
    return np.ascontiguousarray(w.reshape(k // 128, 128, o).transpose(1, 0, 2))


def _chunk_vec(v, ncol):
    """[C] -> [128, ncol] column-chunks (zero padded)."""
    out = np.zeros((128, ncol), np.float32)
    for c in range(ncol):
        seg = v[c * 128 : (c + 1) * 128]
        out[: len(seg), c] = seg
    return out


def _core_rows(d):
    """Slot->global-row map for core d.

    Every core owns one row of each cols class {8,16,...,384}: class
    cols=8k holds global rows [8(k-1)+1 .. 8k] (row i needs
    roundup8(i) = 8k columns); core d takes row 8(k-1)+1+d. The 384
    class has only 7 real rows (377..383); core 7 gets the dummy row 0
    there. Slots are ordered to match the device-side block packing.
    """
    def row_for_k(k):
        i = 8 * (k - 1) + 1 + d
        return 0 if i > M else i

    rows = [0] * R
    for segs in _BLOCKS:
        for slot, _off, ln in segs:
            rows[slot] = row_for_k(ln // 8)
    return rows


def _prep_in_maps(inputs):
    bf = ml_dtypes.bfloat16
    f8 = ml_dtypes.float8_e4m3

    seq = np.asarray(inputs["sequence_output"], np.float32)
    spk = np.asarray(inputs["speaker_emb"], np.float32)
    dummy = np.asarray(inputs["dummy_emb"], np.float32)
    seg = np.asarray(inputs["mentions_seg"]).astype(np.int64)
    mstart = np.asarray(inputs["mention_start"]).astype(np.int64)
    mend = np.asarray(inputs["mention_end"]).astype(np.int64)
    sid = np.asarray(inputs["speaker_ids"]).astype(np.int64)[seg, mstart]
    reps = seq[seg, mstart] + seq[seg, mend] + spk[sid]
    all_reps = np.concatenate([dummy, reps], axis=0)          # [N, H]

    def chunkT(a):
        # [n, H] -> [128, HC, n] transposed layout
        n = a.shape[0]
        return np.ascontiguousarray(
            a.T.reshape(HC, 128, n).transpose(1, 0, 2)
        )

    reps8 = chunkT(all_reps).astype(f8)

    W_pair1 = np.asarray(inputs["W_pair1"], np.float32)
    wa8 = _chunk_cols(np.ascontiguousarray(W_pair1[:, :H].T) * S2).astype(f8)
    wb8 = _chunk_cols(np.ascontiguousarray(W_pair1[:, H:].T) * S2).astype(f8)
    w28 = _chunk_cols(
        np.ascontiguousarray(np.asarray(inputs["W_pair2"], np.float32).T) * S2
    ).astype(f8)
    wm18p = np.zeros((128, HC, 400), np.float32)
    wm18p[:, :, :384] = _chunk_cols(
        np.ascontiguousarray(np.asarray(inputs["W_m1"], np.float32).T) * S2
    )
    wm18p[:, :3, 384] = _chunk_vec(
        np.asarray(inputs["W_pair3"], np.float32)[0] * S3, 3
    ).reshape(128, 3)
    wm18p = wm18p.astype(f8)
    wm2T = _chunk_cols(
        np.ascontiguousarray(np.asarray(inputs["W_m2"], np.float32).T)
    ).astype(bf)
    wm3c = _chunk_vec(np.asarray(inputs["W_m3"], np.float32)[0], 2).astype(bf)
    wc1T = _chunk_cols(
        np.ascontiguousarray(np.asarray(inputs["W_c1"], np.float32).T)
    ).astype(bf)
    wc2T = _chunk_cols(
        np.ascontiguousarray(np.asarray(inputs["W_c2"], np.float32).T)
    ).astype(bf)
    # packed bias columns: b1c(0:6)|b2c(6:9)|bm1c(9:12)|bm2c(12:14)|bc1c(14:17)
    pk128 = np.concatenate([
        _chunk_vec(np.asarray(inputs["b_pair1"], np.float32), HC),
        _chunk_vec(np.asarray(inputs["b_pair2"], np.float32), 3),
        _chunk_vec(np.asarray(inputs["b_m1"], np.float32), 3),
        _chunk_vec(np.asarray(inputs["b_m2"], np.float32), 2),
        _chunk_vec(np.asarray(inputs["b_c1"], np.float32), 3),
    ], axis=1)
    bc2r = np.asarray(inputs["b_c2"], np.float32).reshape(1, 18)

    link_first = np.asarray(inputs["link_first"]).astype(np.int64)
    link_second = np.asarray(inputs["link_second"]).astype(np.int64)
    label = np.asarray(inputs["character_label"]).astype(np.int64)

    mult = np.zeros((N, N), np.float32)
    np.add.at(mult, (link_second, link_first), 1.0)
    has_link = mult.sum(axis=1) > 0
    wnll_full = ((np.arange(N) >= 1) & has_link).astype(np.float32)
    mult[~has_link, 0] = 1.0  # keep log(E) finite; weight is 0 there

    # S3-scaled mask: scores stay S3-scaled until the softmax exp
    mask_full = np.where(
        np.arange(N)[None, :] >= np.arange(N)[:, None],
        np.float32(NEG * S3), 0.0,
    ).astype(np.float32)

    oneh_full = np.zeros((N, 18), np.float32)
    wch_full = np.zeros(N, np.float32)
    oneh_full[np.arange(1, N), label] = 1.0
    wch_full[1:] = 1.0

    shared = dict(
        reps8=reps8, wa8=wa8, wb8=wb8, w28=w28, wm18p=wm18p,
        wm2T=wm2T, wm3c=wm3c, wc1T=wc1T, wc2T=wc2T, pk128=pk128,
    )
    in_maps = []
    for d in range(NC_):
        rows = _core_rows(d)
        m = dict(shared)
        m["repsTl8"] = chunkT(all_reps[rows]).astype(f8)
        # maskb|multb|oneh|wnll|wch packed per-slot; bc2r on row 0
        pk48 = np.zeros((R, 806), np.float32)
        pk48[:, 0:384] = mask_full[rows]
        pk48[:, 384:768] = mult[rows]
        pk48[:, 768:786] = oneh_full[rows]
        pk48[:, 786] = wnll_full[rows]
        pk48[:, 787] = wch_full[rows]
        pk48[0, 788:806] = bc2r[0]
        m["pk48"] = pk48
        in_maps.append(m)
    return in_maps


def kernel(**inputs):
    global LAST_RESULT
    in_maps = _prep_in_maps(inputs)

    # the fused h2 relu drops the (per-spec zero) b_pair2 bias
    fuse = not np.any(np.asarray(inputs["b_pair2"], np.float32))
    key = ("nc", fuse)
    if key not in _CACHE:
        _CACHE[key] = _build_program(fuse_relu=fuse)
    nc = _CACHE[key]

    res = run_bass_kernel_spmd(nc, in_maps, core_ids=list(range(NC_)))
    LAST_RESULT = res
    total = np.float32(0.0)
    for d in range(NC_):
        total += np.float32(res.results[d]["loss"][0, 0])
    return np.asarray(total, dtype=np.float32)


if __name__ == "__main__":
    import reference

    inputs = {k: np.asarray(v) for k, v in reference.setup_inputs().items()}
    out = kernel(**inputs)
    print("kernel out:", out)
